# revision 44
# baseline (speedup 1.0000x reference)
"""Multi-head factorized dense attention on 8 TRN2 NeuronCores.

Reference computation (per batch b):
    V = x @ Wv                      (4096, 256)
    l = x @ Wl, r = x @ Wr          (4096, 64) each
    attn[n, p*64+q] = l[n,p]*r[n,q] (4096, 4096)
    score = softmax(attn, -1)
    o = score @ V                   (shared across heads == plain matmul)
    out = o @ Wo

Sharding: 8 cores = 2 batches x 4 query-row chunks of 1024 rows.

Small O(S*D^2) projections (l, r, V, row-max stats, the final Wo
projection and 1/Z normalize) run on the host; the device does all the
O(S*S) work: outer product, exp, and score @ V.

Device pipeline per 128-row query tile:
    outer product l x r in fp16 (DVE half / Pool half)
    -> exp with (rowmax - 5.4) bias -> E in fp8e4 (ACT), so the softmax
       top lands at e^5.4 = 221 < 240 (TRN e4m3 max) and the window
       reaches ~12 nats down before subnormal flush
    -> one XBAR DMA transpose per 2048-col half on the fp16 BITCAST of
       E8: moves fp8 byte PAIRS, so partitions hold m-pairs (2*p2+c) at
       half the element count
    -> fp8 DoubleRow matmuls with V8/dV8 (residual pair) as the
       STATIONARY weights in standard [p2, c, d] layout and the
       interleaved E^T bytes as the MOVING operand (walrus rejects
       interleaved ldweights APs), producing o^T [d, n]; an all-ones
       (1/64-scaled) weights block accumulates the exact row sum Z of
       the quantized E (PE, 0.5 cyc/row)
    -> ship unnormalized o^T plus Z in fp16; host divides and applies Wo
"""

import sys

sys.path.insert(0, "/opt/trn_rl_repo")

import numpy as np
import ml_dtypes

B, S, D = 2, 4096, 256
PD = 64  # proj_dim_l == proj_dim_r == 64, PD*PD == S
NQ = S // 4  # query rows per core
QT = NQ // 128  # query tiles per core (8)
NJ = S // 256  # DoubleRow j-blocks (16)
N_CORES = 8
BIAS_OFF = 5.4  # exp bias: top element = e^BIAS_OFF = 221 < 240 (e4m3 max)
ZSCALE = 1.0 / 64.0  # ones-column scale: keeps Z within fp16 range

F8NP = ml_dtypes.float8_e4m3

_CACHE = {}


def _build(nloop=0):
    if ("nc", nloop) in _CACHE:
        return _CACHE[("nc", nloop)]

    import concourse.bass as bass
    import concourse.bacc as bacc
    import concourse.tile as tile
    from concourse import mybir

    F32 = mybir.dt.float32
    F16 = mybir.dt.float16
    F8 = mybir.dt.float8e4
    EXP = mybir.ActivationFunctionType.Exp
    DR = mybir.MatmulPerfMode.DoubleRow

    nc = bacc.Bacc("TRN2", target_bir_lowering=False, debug=False)

    lr_d = nc.dram_tensor("lr16", [128, QT, 128], F16, kind="ExternalInput").ap()
    # tile-0 l/r plus all tiles' negmx (fp16: the bias rounding is uniform
    # per row, so it cancels exactly in the softmax normalization)
    lrh_d = nc.dram_tensor("lrh16", [128, 136], F16, kind="ExternalInput").ap()
    v8_d = nc.dram_tensor(
        "v8", [128, NJ, 2, 2, 2, 128], F8, kind="ExternalInput"
    ).ap()
    ones_d = nc.dram_tensor("ones8", [128, 2, 128], F8, kind="ExternalInput").ap()
    out_d = nc.dram_tensor("out", [NQ, 3 * 128], F16, kind="ExternalOutput").ap()

    with tile.TileContext(nc) as tc:
        import contextlib

        with contextlib.ExitStack() as ctx:
            if nloop:
                ctx.enter_context(tc.For_i(0, nloop, 1))
            persist = ctx.enter_context(tc.tile_pool(name="persist", bufs=1))
            prodp = ctx.enter_context(tc.tile_pool(name="prodp", bufs=4))
            ep = ctx.enter_context(tc.tile_pool(name="ep", bufs=3))
            etp = ctx.enter_context(tc.tile_pool(name="etp", bufs=6))
            work = ctx.enter_context(tc.tile_pool(name="work", bufs=2))
            psO = ctx.enter_context(tc.tile_pool(name="psO", bufs=2, space="PSUM"))

            lr16a = persist.tile([128, 136], F16, tag="lr16a")
            lr16 = persist.tile([128, QT, 128], F16, tag="lr16")
            v8 = persist.tile([128, NJ, 2, 2, 2, 128], F8, tag="v8")
            ones8 = persist.tile([128, 2, 128], F8, tag="ones8")
            dmy = persist.tile([128, 1], F16, tag="dmy")
            dmy8 = persist.tile([128, 1], F8, tag="dmy8")

            # warm the ACT exp table while loads run
            nc.vector.memset(dmy[:], 0.0)
            nc.scalar.activation(out=dmy8[:], in_=dmy[:], func=EXP, bias=0.0, scale=1.0)

            prod_t = {}
            e8_t = {}
            et_t = {}
            ops_t = {}

            def outer(t, chunks=((0, 32, "v"), (32, 64, "g"))):
                if t == 0:
                    l_ap = lr16a[:, 0:PD]
                    r_ap = lr16a[:, PD : 2 * PD]
                else:
                    l_ap = lr16[:, t, 0:PD]
                    r_ap = lr16[:, t, PD : 2 * PD]
                prod = prodp.tile([128, PD, PD], F16, tag="prod", name=f"prod{t}")
                prod_t[t] = prod
                for p0, p1, eng in chunks:
                    w = p1 - p0
                    l_b = l_ap[:, p0:p1].broadcast_to([128, w, PD])
                    r_b = bass.AP(
                        tensor=r_ap.tensor,
                        offset=r_ap.offset,
                        ap=[r_ap.ap[0], [0, w], r_ap.ap[1]],
                    )
                    e = nc.vector if eng == "v" else nc.gpsimd
                    e.tensor_mul(prod[:, p0:p1, :], l_b, r_b)

            def expf(t, widths=(S,)):
                E8 = ep.tile([128, S], F8, tag="E8", name=f"E8{t}")
                e8_t[t] = E8
                pflat = prod_t[t][:].rearrange("p a b -> p (a b)")
                c0 = 0
                for w in widths:
                    nc.scalar.activation(
                        out=E8[:, c0 : c0 + w],
                        in_=pflat[:, c0 : c0 + w],
                        func=EXP,
                        bias=lr16a[:, 128 + t : 129 + t],
                        scale=1.0,
                    )
                    c0 += w
                assert c0 == S

            def xbar(t, j0, nj):
                E16 = e8_t[t][:].bitcast(F16)  # [128, 2048]
                half = bass.AP(
                    tensor=E16.tensor,
                    offset=E16.offset + j0 * 128,
                    ap=[E16.ap[0], [1, nj * 128]],
                )
                et = etp.tile([128, nj, 128], F16, tag="et", name=f"et{t}_{j0}")
                et_t[(t, j0)] = et
                nc.sync.dma_start(out=et[:], in_=half, transpose=True)

            def back(t, j0, nj):
                if j0 == 0:
                    ops_t[t] = [
                        psO.tile([128, 128], F32, tag=f"ops{i}", name=f"ops{i}_{t}")
                        for i in range(3)
                    ]
                ops = ops_t[t]
                et8 = et_t[(t, j0)][:].bitcast(F8)
                for jj in range(nj):
                    j = j0 + jj
                    # moving operand: interleaved (c, n) bytes of E^T pairs
                    x = bass.AP(
                        tensor=et8.tensor,
                        offset=et8.offset + jj * 256,
                        ap=[et8.ap[0], [1, 2], [2, 128]],
                    )
                    for w in range(2):
                        for h in range(2):
                            nc.tensor.matmul(
                                ops[h][:],
                                v8[:, j, w, :, h, :],
                                x,
                                start=(j == 0 and w == 0),
                                stop=(j == NJ - 1 and w == 1),
                                perf_mode=DR,
                            )
                    nc.tensor.matmul(
                        ops[2][:],
                        ones8[:],
                        x,
                        start=(j == 0),
                        stop=(j == NJ - 1),
                        perf_mode=DR,
                    )

            def epi(t, use_act=False):
                tsl = slice(t * 128, (t + 1) * 128)
                ops = ops_t[t]
                osb = work.tile([128, 3, 128], F16, tag="osb", name=f"osb{t}")
                for i in range(3):
                    if use_act and i == 2:
                        # ACT is idle in the tail; offload one PSUM copy
                        nc.scalar.activation(
                            out=osb[:, i, :],
                            in_=ops[i][:],
                            func=mybir.ActivationFunctionType.Copy,
                        )
                    else:
                        nc.vector.tensor_copy(osb[:, i, :], ops[i][:])
                nc.sync.dma_start(out=out_d[tsl, :], in_=osb[:])

            # ---- loads + software pipeline ----
            # SP queue order: lr16[t0], negmx, lr16[rest], v8(first half),
            # xbar(0,*), v8(second half), xbar(1,*)...
            nc.sync.dma_start(out=lr16a, in_=lrh_d)
            nc.sync.dma_start(out=lr16[:, 1:QT, :], in_=lr_d[:, 1:QT, :])
            nc.sync.dma_start(out=ones8, in_=ones_d)
            nc.sync.dma_start(out=v8[:, 0 : NJ // 2], in_=v8_d[:, 0 : NJ // 2])

            # tile 0: fine-grained start so ACT begins ASAP
            outer(
                0,
                chunks=((0, 8, "g"), (8, 16, "g"), (16, 32, "g"), (32, 64, "v")),
            )
            expf(0, widths=(512, 512, 1024, 2048))
            outer(1)
            expf(1)
            xbar(0, 0, 8)
            xbar(0, 8, 8)
            nc.sync.dma_start(out=v8[:, NJ // 2 : NJ], in_=v8_d[:, NJ // 2 : NJ])
            back(0, 0, 8)
            back(0, 8, 8)
            for t in range(2, QT):
                outer(t)
                expf(t, widths=(2048, 1024, 1024) if t == QT - 1 else (S,))
                epi(t - 2)
                xbar(t - 1, 0, 8)
                back(t - 1, 0, 8)
                xbar(t - 1, 8, 8)
                back(t - 1, 8, 8)
            epi(QT - 2)
            # tile 7 tail: progressively smaller units so the last
            # exp->xbar->matmul chain is short
            xbar(QT - 1, 0, 8)
            back(QT - 1, 0, 8)
            xbar(QT - 1, 8, 4)
            back(QT - 1, 8, 4)
            xbar(QT - 1, 12, 4)
            back(QT - 1, 12, 4)
            epi(QT - 1, use_act=True)

    nc.compile()
    _CACHE[("nc", nloop)] = nc
    return nc


def _in_maps(x, Wl, Wr, Wv, Wo):
    x = np.ascontiguousarray(x, np.float32)

    v8s = []
    l16s = []
    r16s = []
    ones8 = np.full((128, 2, 128), ZSCALE, F8NP)
    for b in range(B):
        V = x[b] @ np.asarray(Wv, np.float32)
        V8 = V.astype(F8NP)
        dV8 = (V - V8.astype(np.float32)).astype(F8NP)
        v8 = np.zeros((128, NJ, 2, 2, 2, 128), F8NP)
        p2 = np.arange(128)
        for j in range(NJ):
            for c in range(2):
                rows = j * 256 + 2 * p2 + c
                for h in range(2):
                    v8[:, j, 0, c, h, :] = V8[rows, h * 128 : (h + 1) * 128]
                    v8[:, j, 1, c, h, :] = dV8[rows, h * 128 : (h + 1) * 128]
        v8s.append(v8)
        l16s.append((x[b] @ np.asarray(Wl, np.float32)).astype(np.float16))
        r16s.append((x[b] @ np.asarray(Wr, np.float32)).astype(np.float16))

    maps = []
    for c in range(N_CORES):
        b, q = c // 4, (c % 4) * NQ
        l16 = l16s[b][q : q + NQ].reshape(QT, 128, PD)
        r16 = r16s[b][q : q + NQ].reshape(QT, 128, PD)
        lr16 = np.concatenate([l16, r16], axis=2).transpose(1, 0, 2)
        lf = l16.astype(np.float32)
        rf = r16.astype(np.float32)
        corners = np.stack(
            [
                lf.max(2) * rf.max(2),
                lf.max(2) * rf.min(2),
                lf.min(2) * rf.max(2),
                lf.min(2) * rf.min(2),
            ],
            axis=0,
        ).max(0)
        # device prod is fp16(l*r); max of rounded == round(max) (monotone)
        mx16 = corners.astype(np.float16).astype(np.float32)  # [QT, 128]
        negmx = (-mx16 + np.float32(BIAS_OFF)).T.astype(np.float16)  # [128, QT]
        lrh = np.concatenate([lr16[:, 0, :], negmx], axis=1)  # [128, 136]
        maps.append(
            {
                "lr16": np.ascontiguousarray(lr16),
                "lrh16": np.ascontiguousarray(lrh),
                "v8": v8s[b],
                "ones8": ones8,
            }
        )
    return maps


def _finish(res_core, Wo):
    """Host epilogue for one core: reassemble o^T, normalize by Z, apply Wo."""
    arr = res_core.astype(np.float32)  # [NQ, 384]: per-tile o^T halves + Z rows
    out = np.empty((NQ, D), np.float32)
    for t in range(QT):
        blk = arr[t * 128 : (t + 1) * 128]
        o_un = np.concatenate([blk[:, 0:128].T, blk[:, 128:256].T], axis=1)
        Z = blk[0, 256:384] / np.float32(ZSCALE)
        out[t * 128 : (t + 1) * 128] = o_un / Z[:, None]
    return out @ np.asarray(Wo, np.float32)


def kernel(x, Wl, Wr, Wv, Wo, _trace=False, _result_holder=None):
    from concourse.bass_utils import run_bass_kernel_spmd

    nc = _build()
    maps = _in_maps(x, Wl, Wr, Wv, Wo)
    res = run_bass_kernel_spmd(nc, maps, list(range(N_CORES)), trace=_trace)
    if _result_holder is not None:
        _result_holder.append(res)
    out = np.empty((B, S, D), np.float32)
    for c in range(N_CORES):
        b, q = c // 4, (c % 4) * NQ
        out[b, q : q + NQ] = _finish(res.results[c]["out"], Wo)
    return out
